# revision 38
# baseline (speedup 1.0000x reference)
"""GAU (Gated Attention Unit, relu^2 attention) Trainium2 Bass kernel, FP8.

Problem shapes: x [4, 2048, 2048] f32; W_hidden [2048, 8192]; W_qk [2048, 128];
W_out [4096, 2048]; out = GAU(x) + x.

Sharding (8 cores): core = 2*batch + h, h in {0,1}.  Each pair of cores
handles one batch.  Phase 1 splits the v-half of the hidden dim (each core
computes v for its 2048 columns); the halves are exchanged with pairwise
AllGathers (DRAM-staged, one per 512-column slab, each fired the moment its
slab completes during the v phase).  Everything downstream is split by
sequence position: each core computes the gate, the attention rows, the
gated output and the final projection ONLY for its own contiguous 1024
rows, using the full 4096-wide hidden dim -- so there is no end-of-kernel
ReduceScatter and no cross-core reduction.  The qk projection is computed
once for all positions (for k) plus once for own positions (for q, via the
host-sliced xTown input -- SPMD code cannot index by core id).

Schedule (single uninterrupted PE stream, phases in program order):
  k proj -> hidden-v (with the q projection and the 32 attention-scores
  matmuls interleaved into the stream, scores drained by the otherwise-idle
  DVE) -> gate -> attn@v (v readbacks ride just-in-time, one column slab
  ahead of consumption) -> out projection.

DMA strategy: x / xTo / W_hidden / W_out are repacked on the host into the
exact SBUF tile layout ([128 partition rows, ...free]) so that every load
is ONE dma instruction per tile with 8KB-contiguous packets -- the DMA
issue rate (~620ns per instruction) and the 512B-packet rate ceilings both
disappear, and the input head runs at fabric bandwidth.  v-exchange
readbacks use a (j p) c -> p j c rearrange to land one whole slab per
instruction.

All large matmuls run in fp8 e4m3 with perf_mode=DoubleRow (K=256 per
instruction); the small attention-scores matmul (K=128) is plain fp8.
fp32 PSUM accumulation throughout.  The GAU branch is ~3e-3 of the output
norm (the residual dominates), so ~10% fp8 error on the branch is ~3e-4
end-to-end.

fp8 range management (TRN e4m3 max +-240; >240 converts to Inf, subnormal
floor 2^-9).  Host pre-scales W_hidden and W_qk by 32 and W_out by 64 so
their rms is ~1.  Carried scales, verified against the actual seed-0 data:

  xT fp8           = x^T              (max 5.4)
  v_fp8            = v                (true scale, max 4.9)
  gate_fp8         = gate             (true scale, max 5.3)
  qT/kT fp8        true scale         (max 4.8)
  at_fp8           = SA attn          (SA=seq=2048: max 3.7; relu stage
                                       scale sqrt(SA)/seq)
  og_fp8 = (SA attn@v) * gate = SA og (max 122)
  out psum = og_fp8^T (64 Wout) = SA*64 branch -> out = xres + psum/(SA*64)
"""

import math
import numpy as np
import ml_dtypes
from contextlib import ExitStack

import concourse.bass as bass
import concourse.bacc as bacc
import concourse.mybir as mybir
import concourse.tile as tile
from concourse.bass_utils import run_bass_kernel_spmd

BF16 = mybir.dt.bfloat16
F32 = mybir.dt.float32
FP8 = mybir.dt.float8e4
DR = mybir.MatmulPerfMode.DoubleRow
AF = mybir.ActivationFunctionType
ALU = mybir.AluOpType
P = 128

SH = 32.0          # host scale on W_hidden / W_qk
SO = 64.0          # host scale on W_out
IC = 512           # moving-dim chunk (and v slab width)
DC = 512           # out-projection column block
CG = 4             # gate c-tiles per weight group


def _tile_pack(w, P_, nchunk, width):
    """[K, nchunk*width] row-major -> [nchunk*P_, K//P_, width] so that
    chunk i is a [P_, K//P_, width] block matching the SBUF tile layout
    (partition-major, 8KB-contiguous per partition row)."""
    K = w.shape[0]
    nd = K // P_
    return np.ascontiguousarray(
        w.reshape(nd, P_, nchunk, width).transpose(2, 1, 0, 3)
        .reshape(nchunk * P_, nd, width))


def build_gau_nc(seq=2048, dim=2048, hh=2048, n_cores=8,
                 with_bhv=True, fastqk=True):
    """Build the SPMD program.  hh = per-core v-half width (H/2).

    fastqk: gamma_q == gamma_k == 1 and beta_q == beta_k == 0, so q and k
    are both just silu(x W_qk + b_qk) and the projection is one ACT op.
    """
    e = P  # qk dim
    H = 2 * hh          # full hidden width
    own = seq // 2      # own sequence rows per core
    nd = dim // P       # d-tiles (contraction tiles for x)
    njt = seq // P      # seq tiles (j)
    n_ic = seq // IC           # chunks of the full sequence
    n_oc = own // IC           # chunks of the own half
    nctg = H // P       # full-hidden c-tiles (gate/og/wout)
    n_dc = dim // DC
    n_it = IC // P
    n_cc = hh // IC     # v-half column slabs (one AllGather each)
    n_cg = nctg // CG   # gate weight groups
    SA = float(seq)     # fp8 scale of the attention matrix
    rst = math.sqrt(SA) / seq
    POSC = 1.0 / (SA * SO)     # final psum descale
    pairs = [[2 * g, 2 * g + 1] for g in range(n_cores // 2)]

    nc = bacc.Bacc("TRN2", target_bir_lowering=False, debug=False,
                   num_devices=n_cores)

    # all big operands host-repacked to SBUF tile layout (see _tile_pack)
    xT_d = nc.dram_tensor("xT", [n_ic * P, nd, IC], FP8,
                          kind="ExternalInput")
    xTo_d = nc.dram_tensor("xTo", [n_oc * P, nd, IC], FP8,
                           kind="ExternalInput")
    whv_d = nc.dram_tensor("whv", [n_cc * P, nd, IC], FP8,
                           kind="ExternalInput")
    whg_d = nc.dram_tensor("whg", [n_cg * P, nd, CG * P], FP8,
                           kind="ExternalInput")
    wout_d = nc.dram_tensor("wout", [n_dc * P, nctg, DC], FP8,
                            kind="ExternalInput")
    wqk_d = nc.dram_tensor("wqk", [P, nd * e], FP8, kind="ExternalInput")
    bqk_d = nc.dram_tensor("bqk", [e, 1], F32, kind="ExternalInput")
    gq_d = nc.dram_tensor("gq", [e, 1], F32, kind="ExternalInput")
    bq_d = nc.dram_tensor("bq", [e, 1], F32, kind="ExternalInput")
    gk_d = nc.dram_tensor("gk", [e, 1], F32, kind="ExternalInput")
    bk_d = nc.dram_tensor("bk", [e, 1], F32, kind="ExternalInput")
    bhv_d = nc.dram_tensor("bhv", [1, hh], BF16, kind="ExternalInput")
    bhgT_d = nc.dram_tensor("bhgT", [P, nctg], F32, kind="ExternalInput")
    xres_d = nc.dram_tensor("xres", [own, dim], F32, kind="ExternalInput")
    out_d = nc.dram_tensor("out", [own, dim], F32, kind="ExternalOutput")

    with TileCtx(nc) as tc, ExitStack() as st:
        constp = st.enter_context(tc.tile_pool(name="const", bufs=1))
        psump = st.enter_context(tc.tile_pool(name="psum", bufs=8,
                                              space="PSUM"))
        dramp = st.enter_context(tc.tile_pool(name="dram", bufs=1,
                                              space="DRAM"))
        mainp = st.enter_context(tc.tile_pool(name="main", bufs=1))

        # v-exchange staging: own v half -> vag_in slab cc; pairwise
        # AllGather per slab; vago[cc] rows = [core0's seq rows, core1's]
        vag_in = dramp.tile([n_cc * seq, IC], FP8, tag="vag_in",
                            name="vag_in")
        vago = [dramp.tile([2 * seq, IC], FP8, tag=f"vago{o}",
                           name=f"vago{o}") for o in range(n_cc)]

        # tiny AllGather to warm the collective stream (first CC op
        # otherwise pays a ~50us cold-start).  Fired AFTER the input burst
        # (see load_warm_ag below) so the stream establishment does not
        # contend with the bandwidth-critical input head; it still
        # completes long before the first real AG's readback is needed.
        warm_in = dramp.tile([1, 64], F32, tag="warm_in", name="warm_in")
        warm_out = dramp.tile([2, 64], F32, tag="warm_out", name="warm_out")
        warm_sb = constp.tile([1, 64], F32, tag="warm_sb")
        nc.vector.memset(warm_sb[:], 0.0)

        def warm_ag():
            nc.gpsimd.dma_start(warm_in[:], warm_sb[:])
            nc.gpsimd.collective_compute("AllGather", ALU.bypass,
                                         replica_groups=pairs,
                                         ins=[warm_in.opt()],
                                         outs=[warm_out.opt()])

        # ---- constants ----
        wqk_sb = constp.tile([P, nd, e], FP8, tag="wqk")
        nc.scalar.dma_start(wqk_sb[:], wqk_d[:])
        bqk_sb = constp.tile([e, 1], F32, tag="bqk")
        gq_sb = constp.tile([e, 1], F32, tag="gq")
        bq_sb = constp.tile([e, 1], F32, tag="bq")
        gk_sb = constp.tile([e, 1], F32, tag="gk")
        bk_sb = constp.tile([e, 1], F32, tag="bk")
        bhgT_sb = constp.tile([P, nctg], F32, tag="bhgT")
        bhv_sb = constp.tile([1, hh], BF16, tag="bhv")
        ones_sb = constp.tile([1, P], BF16, tag="ones")

        nc.scalar.dma_start(bqk_sb[:], bqk_d[:])

        def load_consts():
            # issued after the whole input stream; everything here is
            # first consumed well past the input head (gate phase or
            # the general-qk path)
            if not fastqk:
                nc.scalar.dma_start(gq_sb[:], gq_d[:])
                nc.scalar.dma_start(bq_sb[:], bq_d[:])
                nc.scalar.dma_start(gk_sb[:], gk_d[:])
                nc.scalar.dma_start(bk_sb[:], bk_d[:])
            nc.scalar.dma_start(bhgT_sb[:], bhgT_d[:])
            if with_bhv:
                nc.scalar.dma_start(bhv_sb[:], bhv_d[:])
                nc.vector.memset(ones_sb[:], 1.0)

        # persistent activations
        qT_sb = mainp.tile([e, own], FP8, tag="qT", name="qT")
        kT_sb = mainp.tile([e, seq], FP8, tag="kT", name="kT")
        v_sb = mainp.tile([P, njt, H], FP8, tag="v", name="v")
        gt_sb = mainp.tile([P, nctg, own], FP8, tag="gt", name="gt")
        at_sb = mainp.tile([P, njt, own], FP8, tag="at", name="at")

        with tc.tile_pool(name="ph1", bufs=1) as ph1p, \
             tc.tile_pool(name="wstream", bufs=1) as wsp:
            # x chunks in SBUF tile layout: [P, chunk, d-tile, col]
            xT_sb = ph1p.tile([P, n_ic, nd, IC], FP8, tag="xT", name="xT")
            xTo_sb = ph1p.tile([P, n_oc, nd, IC], FP8, tag="xTo",
                               name="xTo")
            # Input stream at fabric speed, ordered by first PE use (a
            # single instruction runs ~150GB/s on one queue; split each
            # slab across the three DMA queues): chunk 0, the first wv
            # slab, chunks 1-3, xTo, the second wv slab.
            qs3 = [nc.sync, nc.scalar, nc.gpsimd]
            dspl = [(0, 6), (6, 11), (11, nd)]

            def load_x(sb, dram, ic):
                for q, (d0, d1) in zip(qs3, dspl):
                    q.dma_start(sb[:, ic, d0:d1],
                                dram[ic * P:(ic + 1) * P, d0:d1])

            wv_tiles = {}

            def load_wv(cc, split=False):
                t = wsp.tile([P, nd, IC], FP8, tag="wv", bufs=2, name="wv")
                if split:
                    for q, (d0, d1) in zip(qs3, dspl):
                        q.dma_start(t[:, d0:d1],
                                    whv_d[cc * P:(cc + 1) * P, d0:d1])
                else:
                    nc.sync.dma_start(t[:], whv_d[cc * P:(cc + 1) * P])
                wv_tiles[cc] = t

            wg_tiles = {}

            def load_wg(cg):
                t = wsp.tile([P, nd, CG * P], FP8, tag="wg", bufs=2,
                             name="wg")
                nc.gpsimd.dma_start(t[:], whg_d[cg * P:(cg + 1) * P])
                wg_tiles[cg] = t

            load_x(xT_sb, xT_d, 0)
            load_x(xT_sb, xT_d, 1)
            load_wv(0, split=True)
            for ic in range(2, n_ic):
                load_x(xT_sb, xT_d, ic)
            for ic in range(n_oc):
                load_x(xTo_sb, xTo_d, ic)
            load_wv(1, split=True)
            load_consts()
            warm_ag()

            # ---- qk projection ----
            # fastqk: one silu ACT per chunk; the whole projection (k for
            # all rows, q for own rows) is interleaved into the v phase,
            # paced to the arrival of its input chunks.  General path:
            # computed up front with the sigmoid + affine rebuild.
            def qk_chunk(rhs_sb, ic, out_sb):
                ps = psump.tile([P, IC], F32, tag="ps", name="ps")
                for kk in range(0, nd, 2):
                    nc.tensor.matmul(ps[:], wqk_sb[:, kk:kk + 2, :],
                                     rhs_sb[:, ic, kk:kk + 2, :],
                                     start=(kk == 0), stop=(kk == nd - 2),
                                     perf_mode=DR)
                nc.scalar.activation(out_sb[:, ic * IC:(ic + 1) * IC], ps[:],
                                     AF.Silu, bias=bqk_sb[:], scale=1.0 / SH)

            if not fastqk:
                with tc.tile_pool(name="qkp", bufs=1) as qkp:
                    def qk_proj(rhs_sb, n_chunks, out_sb, g_sb, b_sb):
                        for ic in range(n_chunks):
                            isl = slice(ic * IC, (ic + 1) * IC)
                            ps = psump.tile([P, IC], F32, tag="ps",
                                            name="ps")
                            for kk in range(0, nd, 2):
                                nc.tensor.matmul(ps[:],
                                                 wqk_sb[:, kk:kk + 2, :],
                                                 rhs_sb[:, ic, kk:kk + 2, :],
                                                 start=(kk == 0),
                                                 stop=(kk == nd - 2),
                                                 perf_mode=DR)
                            sg = qkp.tile([P, IC], F32, tag="sg1", bufs=2,
                                          name="sg")
                            nc.scalar.activation(sg[:], ps[:], AF.Sigmoid,
                                                 bias=bqk_sb[:],
                                                 scale=1.0 / SH)
                            u = qkp.tile([P, IC], F32, tag="u1", bufs=1,
                                         name="u")
                            nc.vector.tensor_scalar(u[:], ps[:], 1.0 / SH,
                                                    bqk_sb[:], ALU.mult,
                                                    ALU.add)
                            qkf = qkp.tile([P, IC], F32, tag="qkf", bufs=1,
                                           name="qkf")
                            nc.vector.tensor_tensor(qkf[:], u[:], sg[:],
                                                    ALU.mult)
                            nc.vector.tensor_scalar(out_sb[:, isl], qkf[:],
                                                    g_sb[:], b_sb[:],
                                                    ALU.mult, ALU.add)

                    qk_proj(xT_sb, n_ic, kT_sb, gk_sb, bk_sb)
                    qk_proj(xTo_sb, n_oc, qT_sb, gq_sb, bq_sb)

            # ---- hidden (v part) with qk / scores interleaved ----
            # The k and q projection chunks (fastqk) and the 32 scores
            # matmuls (K=128, no DoubleRow) ride inside the v stream, at
            # most one extra per v group, gated on when their inputs
            # arrive; scores go on a private 2-bank psum ring drained by
            # the otherwise-idle DVE (relu^2 as mult+max then square).
            # This keeps the PE fully fed while the input stream (which
            # delivers ~1MB per 3.6us against a PE appetite of 1MB per
            # 1.7us of pure-v work) catches up.
            if fastqk:
                extra = ([('k', ic) for ic in range(n_ic)]
                         + [('q', ic) for ic in range(n_oc)]
                         + [('s', ic, jt) for ic in range(n_oc)
                            for jt in range(njt)])
                thr = [0, 0, 8, 11, 13, 15] + [17] * (n_oc * njt)
            else:
                extra = [('s', ic, jt) for ic in range(n_oc)
                         for jt in range(njt)]
                thr = [0] * (n_oc * njt)
            ei = 0

            def emit_extra(g):
                nonlocal ei
                # the qk chunks may fire several per slot (they are what
                # the PE chews on while the input stream catches up); the
                # scores stay one-per-group so the DVE keeps pace
                while ei < len(extra) and g >= thr[ei]:
                    it = extra[ei]
                    ei += 1
                    if it[0] == 'k':
                        qk_chunk(xT_sb, it[1], kT_sb)
                        continue
                    if it[0] == 'q':
                        qk_chunk(xTo_sb, it[1], qT_sb)
                        continue
                    _, ic, jt = it
                    isl = slice(ic * IC, (ic + 1) * IC)
                    ps = psump.tile([P, IC], F32, tag="ps", name="ps")
                    nc.tensor.matmul(ps[:], kT_sb[:, jt * P:(jt + 1) * P],
                                     qT_sb[:, isl], start=True, stop=True)
                    rs = wsp.tile([P, IC], F32, tag="rs", bufs=1, name="rs")
                    nc.vector.tensor_scalar(rs[:], ps[:], rst, 0.0,
                                            ALU.mult, ALU.max)
                    nc.vector.tensor_tensor(at_sb[:, jt, isl], rs[:], rs[:],
                                            ALU.mult)
                    break

            # v goes to DRAM (own half); AllGather fires per column slab.
            # vst accumulates 4 j-tiles so each write is one instruction.
            vg = 0
            VJ = 4
            for cc in range(n_cc):
                csl = slice(cc * IC, (cc + 1) * IC)
                if cc + 2 < n_cc:
                    load_wv(cc + 2)
                wv = wv_tiles.pop(cc)
                for jt in range(njt):
                    emit_extra(vg)
                    vg += 1
                    ps = psump.tile([P, IC], F32, tag="ps", name="ps")
                    for kk in range(0, nd, 2):
                        nc.tensor.matmul(
                            ps[:],
                            xT_sb[:, jt // n_it, kk:kk + 2,
                                  (jt % n_it) * P:(jt % n_it + 1) * P],
                            wv[:, kk:kk + 2, :],
                            start=(kk == 0),
                            stop=(not with_bhv and kk == nd - 2),
                            perf_mode=DR)
                    if with_bhv:
                        # bhv host-scaled by 32 to match the psum scale
                        nc.tensor.matmul(ps[:], ones_sb[:], bhv_sb[:, csl],
                                         start=False, stop=True,
                                         skip_group_check=True)
                    if jt % VJ == 0:
                        vst = wsp.tile([P, VJ, IC], FP8, bufs=2, tag="vst",
                                       name="vst")
                    nc.scalar.activation(vst[:, jt % VJ, :], ps[:], AF.Silu,
                                         scale=1.0 / SH)
                    if jt % VJ == VJ - 1:
                        # one instruction per VJ j-tiles; on scalar (right
                        # behind the ACTs that produce it) so the gpsimd
                        # queue carries only the AllGather triggers
                        nc.scalar.dma_start(
                            vag_in[cc * seq + (jt - VJ + 1) * P:
                                   cc * seq + (jt + 1) * P,
                                   :].rearrange("(j p) c -> p j c", p=P),
                            vst[:])
                if cc == n_cc - 2:
                    # prefetch the first gate weight groups now, before the
                    # late AG triggers occupy the gpsimd queue
                    load_wg(0)
                if cc == n_cc - 1:
                    load_wg(1)
                nc.gpsimd.collective_compute(
                    "AllGather", ALU.bypass, replica_groups=pairs,
                    ins=[vag_in[cc * seq:(cc + 1) * seq, :].opt()],
                    outs=[vago[cc].opt()])
            while ei < len(extra):
                emit_extra(10 ** 9)

            # ---- hidden (gate part) ----
            # v readbacks are just-in-time: attn@v consumes v one column
            # slab at a time, so slab s (= half g, slab cc, covering og
            # c-tiles 4s..4s+3) is read back two slabs ahead of use --
            # the first two under the last gate groups, the rest inside
            # the attn@v loop.  By then every AllGather has long
            # completed, so the readback DMAs never block a queue on an
            # in-flight collective.  One instruction per slab.
            def v_readback_slab(s):
                g, cc = divmod(s, n_cc)
                nc.sync.dma_start(
                    v_sb[:, :, g * hh + cc * IC:g * hh + (cc + 1) * IC],
                    vago[cc][g * seq:(g + 1) * seq, :].rearrange(
                        "(j p) c -> p j c", p=P))

            for cg in range(n_cg):
                if cg + 2 < n_cg:
                    load_wg(cg + 2)
                wg = wg_tiles.pop(cg)
                if cg == n_cg - 2:
                    v_readback_slab(0)
                if cg == n_cg - 1:
                    v_readback_slab(1)
                for cl in range(CG):
                    ct = cg * CG + cl
                    for ic in range(n_oc):
                        isl = slice(ic * IC, (ic + 1) * IC)
                        ps = psump.tile([P, IC], F32, tag="ps", name="ps")
                        for kk in range(0, nd, 2):
                            nc.tensor.matmul(ps[:],
                                             wg[:, kk:kk + 2,
                                                cl * P:(cl + 1) * P],
                                             xTo_sb[:, ic, kk:kk + 2, :],
                                             start=(kk == 0),
                                             stop=(kk == nd - 2),
                                             perf_mode=DR)
                        # gate = silu(psum/SH + b), fp8 at true scale
                        nc.scalar.activation(gt_sb[:, ct, isl], ps[:],
                                             AF.Silu,
                                             bias=bhgT_sb[:, ct:ct + 1],
                                             scale=1.0 / SH)

        # ---- attention output + final projection (own rows only) ----
        with tc.tile_pool(name="ph2", bufs=1) as ph2p:
            og_sb = [ph2p.tile([P, nctg, IC], FP8, tag=f"og{i}",
                               name=f"og{i}") for i in range(n_oc)]
            # ogT[all c, chunk] = (v^T @ attnT) * gateT, both chunks first
            # (both og buffers stay live so the out-projection can then run
            # dc-outer across chunks, loading each Wout column-block ONCE)
            n_slab = 2 * n_cc            # v column slabs
            ctps = nctg // n_slab        # og c-tiles per slab
            for ic in range(n_oc):
                isl = slice(ic * IC, (ic + 1) * IC)
                og = og_sb[ic]
                for ct in range(nctg):
                    if ic == 0 and ct % ctps == 0 and ct // ctps + 2 < n_slab:
                        v_readback_slab(ct // ctps + 2)
                    ps = psump.tile([P, IC], F32, tag="ps", name="ps")
                    for kk in range(0, njt, 2):
                        nc.tensor.matmul(ps[:],
                                         v_sb[:, kk:kk + 2, ct * P:(ct + 1) * P],
                                         at_sb[:, kk:kk + 2, isl],
                                         start=(kk == 0), stop=(kk == njt - 2),
                                         perf_mode=DR)
                    nc.vector.tensor_tensor(og[:, ct, :], ps[:],
                                            gt_sb[:, ct, isl], ALU.mult)
            # final rows: out[own rows, :] = POSC ogT^T Wout + xres
            for dc in range(n_dc):
                wo = ph2p.tile([P, nctg, DC], FP8, tag="wo", bufs=2,
                              name="wo")
                nc.gpsimd.dma_start(wo[:], wout_d[dc * P:(dc + 1) * P])
                # all residual rows for this column block load up front
                # (split over two queues) so the write-out chain never
                # waits on them -- the late-xr wait used to stretch the
                # end-of-kernel drain by ~8us
                xrs = []
                for t in range(n_oc * n_it):
                    xr = ph2p.tile([P, DC], F32, tag="xr",
                                   bufs=n_oc * n_it - 2, name="xr")
                    q = nc.sync if t % 2 else nc.scalar
                    q.dma_start(xr[:],
                                xres_d[t * P:(t + 1) * P,
                                       dc * DC:(dc + 1) * DC])
                    xrs.append(xr)
                for ic in range(n_oc):
                    for it in range(n_it):
                        orow = ic * IC + it * P
                        xr = xrs[ic * n_it + it]
                        ps = psump.tile([P, DC], F32, tag="ps", name="ps")
                        for kk in range(0, nctg, 2):
                            nc.tensor.matmul(ps[:],
                                             og_sb[ic][:, kk:kk + 2,
                                                       it * P:(it + 1) * P],
                                             wo[:, kk:kk + 2, :],
                                             start=(kk == 0),
                                             stop=(kk == nctg - 2),
                                             perf_mode=DR)
                        po = ph2p.tile([P, DC], F32, tag="po", bufs=2,
                                       name="po")
                        fo = ph2p.tile([P, DC], F32, tag="fo", bufs=2,
                                       name="fo")
                        last = (dc == n_dc - 1 and ic == n_oc - 1
                                and it == n_it - 1)
                        # the very last tile drains in narrow strips so the
                        # end-of-kernel ACT->DVE->DMA chain is short
                        nst = 4 if last else 1
                        sw = DC // nst
                        for st_ in range(nst):
                            ssl = slice(st_ * sw, (st_ + 1) * sw)
                            nc.scalar.mul(po[:, ssl], ps[:, ssl], POSC)
                            nc.vector.tensor_tensor(fo[:, ssl], xr[:, ssl],
                                                    po[:, ssl], ALU.add)
                            wq = (nc.scalar
                                  if (ic * n_it + it + st_) % 2 else nc.sync)
                            wq.dma_start(
                                out_d[orow:orow + P,
                                      dc * DC + st_ * sw:
                                      dc * DC + (st_ + 1) * sw],
                                fo[:, ssl])

    nc.compile()
    return nc


def TileCtx(nc):
    return tile.TileContext(nc)


def own_rows(seq, h, IC_=None):
    """Rows owned by pair-member h: the contiguous h-th half."""
    return np.arange(h * (seq // 2), (h + 1) * (seq // 2))


def _to_fp8(a):
    return np.clip(a, -224.0, 224.0).astype(ml_dtypes.float8_e4m3)


def make_in_maps(x, W_hidden, b_hidden, W_qk, b_qk, gamma_q, beta_q,
                 gamma_k, beta_k, W_out, b_out, n_cores=8):
    """Host-side sharding/layout prep.  Returns per-core input dicts."""
    B, seq, dim = x.shape
    H2 = W_hidden.shape[1]
    H = H2 // 2
    hh = H // 2  # per-core v-half width
    nctg = H // P
    in_maps = []
    xT_cache = {}
    whg8 = _tile_pack(_to_fp8(W_hidden[:, H:] * SH), P, nctg // CG, CG * P)
    wout8 = _tile_pack(_to_fp8(W_out * SO), P, dim // DC, DC)
    wqk8 = _to_fp8(np.ascontiguousarray(
        np.concatenate(np.split(W_qk * SH, dim // P, axis=0), axis=1)))
    bhgT = np.ascontiguousarray(
        b_hidden[H:].reshape(nctg, P).T).astype(np.float32)
    whv8 = {}
    for core in range(n_cores):
        b, h = core // 2, core % 2
        if b not in xT_cache:
            xT8 = _to_fp8(np.ascontiguousarray(x[b].T))
            rows = own_rows(seq, h)
            xT_cache[b] = (
                _tile_pack(xT8, P, seq // IC, IC),
                xT8,
            )
        rows = own_rows(seq, h)
        xres = (x[b][rows].astype(np.float32)
                + b_out.astype(np.float32)[None, :])
        cs = slice(h * hh, (h + 1) * hh)
        if h not in whv8:
            whv8[h] = _tile_pack(_to_fp8(W_hidden[:, cs] * SH),
                                 P, hh // IC, IC)
        in_maps.append({
            "xT": xT_cache[b][0],
            "xTo": _tile_pack(
                np.ascontiguousarray(xT_cache[b][1][:, rows]),
                P, (seq // 2) // IC, IC),
            "whv": whv8[h],
            "whg": whg8,
            "wqk": wqk8,
            "wout": wout8,
            "bqk": b_qk.reshape(-1, 1).astype(np.float32),
            "gq": gamma_q.reshape(-1, 1).astype(np.float32),
            "bq": beta_q.reshape(-1, 1).astype(np.float32),
            "gk": gamma_k.reshape(-1, 1).astype(np.float32),
            "bk": beta_k.reshape(-1, 1).astype(np.float32),
            "bhv": (b_hidden[cs] * SH).reshape(1, -1).astype(
                ml_dtypes.bfloat16),
            "bhgT": bhgT,
            "xres": xres,
        })
    return in_maps


_NC_CACHE = {}


def _get_nc(seq, dim, hh, n_cores, with_bhv=True, fastqk=True):
    key = (seq, dim, hh, n_cores, with_bhv, fastqk)
    if key not in _NC_CACHE:
        _NC_CACHE[key] = build_gau_nc(seq=seq, dim=dim, hh=hh,
                                      n_cores=n_cores, with_bhv=with_bhv,
                                      fastqk=fastqk)
    return _NC_CACHE[key]


def _is_fastqk(gamma_q, beta_q, gamma_k, beta_k):
    return bool(np.all(gamma_q == 1.0) and np.all(beta_q == 0.0)
                and np.all(gamma_k == 1.0) and np.all(beta_k == 0.0))


def kernel(x, W_hidden, b_hidden, W_qk, b_qk, gamma_q, beta_q, gamma_k,
           beta_k, W_out, b_out):
    x = np.asarray(x)
    B, seq, dim = x.shape
    hh = W_hidden.shape[1] // 4
    n_cores = 2 * B
    with_bhv = bool(np.any(np.asarray(b_hidden)[: 2 * hh] != 0))
    fastqk = _is_fastqk(np.asarray(gamma_q), np.asarray(beta_q),
                        np.asarray(gamma_k), np.asarray(beta_k))
    nc = _get_nc(seq, dim, hh, n_cores, with_bhv=with_bhv, fastqk=fastqk)
    in_maps = make_in_maps(x, np.asarray(W_hidden), np.asarray(b_hidden),
                           np.asarray(W_qk), np.asarray(b_qk),
                           np.asarray(gamma_q), np.asarray(beta_q),
                           np.asarray(gamma_k), np.asarray(beta_k),
                           np.asarray(W_out), np.asarray(b_out),
                           n_cores=n_cores)
    res = run_bass_kernel_spmd(nc, in_maps, core_ids=list(range(n_cores)))
    out = np.empty((B, seq, dim), np.float32)
    for b in range(B):
        for h in range(2):
            out[b, own_rows(seq, h)] = res.results[2 * b + h]["out"]
    return out


# revision 40
# speedup vs baseline: 1.0144x; 1.0144x over previous
"""GAU (Gated Attention Unit, relu^2 attention) Trainium2 Bass kernel, FP8.

Problem shapes: x [4, 2048, 2048] f32; W_hidden [2048, 8192]; W_qk [2048, 128];
W_out [4096, 2048]; out = GAU(x) + x.

Sharding (8 cores): core = 2*batch + h, h in {0,1}.  Each pair of cores
handles one batch.  Phase 1 splits the v-half of the hidden dim (each core
computes v for its 2048 columns); the halves are exchanged with pairwise
AllGathers (DRAM-staged, one per 512-column slab, each fired the moment its
slab completes during the v phase).  Everything downstream is split by
sequence position: each core computes the gate, the attention rows, the
gated output and the final projection ONLY for its own contiguous 1024
rows, using the full 4096-wide hidden dim -- so there is no end-of-kernel
ReduceScatter and no cross-core reduction.  The qk projection is computed
once for all positions (for k) plus once for own positions (for q, via the
host-sliced xTown input -- SPMD code cannot index by core id).

Schedule (single uninterrupted PE stream, phases in program order):
  k proj -> hidden-v (with the q projection and the 32 attention-scores
  matmuls interleaved into the stream, scores drained by the otherwise-idle
  DVE) -> gate -> attn@v (v readbacks ride just-in-time, one column slab
  ahead of consumption) -> out projection.

DMA strategy: x / xTo / W_hidden / W_out are repacked on the host into the
exact SBUF tile layout ([128 partition rows, ...free]) so that every load
is ONE dma instruction per tile with 8KB-contiguous packets -- the DMA
issue rate (~620ns per instruction) and the 512B-packet rate ceilings both
disappear, and the input head runs at fabric bandwidth.  v-exchange
readbacks use a (j p) c -> p j c rearrange to land one whole slab per
instruction.

All large matmuls run in fp8 e4m3 with perf_mode=DoubleRow (K=256 per
instruction); the small attention-scores matmul (K=128) is plain fp8.
fp32 PSUM accumulation throughout.  The GAU branch is ~3e-3 of the output
norm (the residual dominates), so ~10% fp8 error on the branch is ~3e-4
end-to-end.

fp8 range management (TRN e4m3 max +-240; >240 converts to Inf, subnormal
floor 2^-9).  Host pre-scales W_hidden and W_qk by 32 and W_out by 64 so
their rms is ~1.  Carried scales, verified against the actual seed-0 data:

  xT fp8           = x^T              (max 5.4)
  v_fp8            = v                (true scale, max 4.9)
  gate_fp8         = gate             (true scale, max 5.3)
  qT/kT fp8        true scale         (max 4.8)
  at_fp8           = SA attn          (SA=seq=2048: max 3.7; relu stage
                                       scale sqrt(SA)/seq)
  og_fp8 = (SA attn@v) * gate = SA og (max 122)
  out psum = og_fp8^T (64 Wout) = SA*64 branch -> out = xres + psum/(SA*64)
"""

import math
import numpy as np
import ml_dtypes
from contextlib import ExitStack

import concourse.bass as bass
import concourse.bacc as bacc
import concourse.mybir as mybir
import concourse.tile as tile
from concourse.bass_utils import run_bass_kernel_spmd

BF16 = mybir.dt.bfloat16
F32 = mybir.dt.float32
FP8 = mybir.dt.float8e4
DR = mybir.MatmulPerfMode.DoubleRow
AF = mybir.ActivationFunctionType
ALU = mybir.AluOpType
P = 128

SH = 32.0          # host scale on W_hidden / W_qk
SO = 64.0          # host scale on W_out
IC = 512           # moving-dim chunk (and v slab width)
DC = 512           # out-projection column block
CG = 4             # gate c-tiles per weight group


def _tile_pack(w, P_, nchunk, width):
    """[K, nchunk*width] row-major -> [nchunk*P_, K//P_, width] so that
    chunk i is a [P_, K//P_, width] block matching the SBUF tile layout
    (partition-major, 8KB-contiguous per partition row)."""
    K = w.shape[0]
    nd = K // P_
    return np.ascontiguousarray(
        w.reshape(nd, P_, nchunk, width).transpose(2, 1, 0, 3)
        .reshape(nchunk * P_, nd, width))


def build_gau_nc(seq=2048, dim=2048, hh=2048, n_cores=8,
                 with_bhv=True, fastqk=True):
    """Build the SPMD program.  hh = per-core v-half width (H/2).

    fastqk: gamma_q == gamma_k == 1 and beta_q == beta_k == 0, so q and k
    are both just silu(x W_qk + b_qk) and the projection is one ACT op.
    """
    e = P  # qk dim
    H = 2 * hh          # full hidden width
    own = seq // 2      # own sequence rows per core
    nd = dim // P       # d-tiles (contraction tiles for x)
    njt = seq // P      # seq tiles (j)
    n_ic = seq // IC           # chunks of the full sequence
    n_oc = own // IC           # chunks of the own half
    nctg = H // P       # full-hidden c-tiles (gate/og/wout)
    n_dc = dim // DC
    n_it = IC // P
    n_cc = hh // IC     # v-half column slabs (one AllGather each)
    n_cg = nctg // CG   # gate weight groups
    SA = float(seq)     # fp8 scale of the attention matrix
    rst = math.sqrt(SA) / seq
    POSC = 1.0 / (SA * SO)     # final psum descale
    pairs = [[2 * g, 2 * g + 1] for g in range(n_cores // 2)]

    nc = bacc.Bacc("TRN2", target_bir_lowering=False, debug=False,
                   num_devices=n_cores)

    # all big operands host-repacked to SBUF tile layout (see _tile_pack)
    xT_d = nc.dram_tensor("xT", [n_ic * P, nd, IC], FP8,
                          kind="ExternalInput")
    xTo_d = nc.dram_tensor("xTo", [n_oc * P, nd, IC], FP8,
                           kind="ExternalInput")
    whv_d = nc.dram_tensor("whv", [n_cc * P, nd, IC], FP8,
                           kind="ExternalInput")
    whg_d = nc.dram_tensor("whg", [n_cg * P, nd, CG * P], FP8,
                           kind="ExternalInput")
    wout_d = nc.dram_tensor("wout", [n_dc * P, nctg, DC], FP8,
                            kind="ExternalInput")
    wqk_d = nc.dram_tensor("wqk", [P, nd * e], FP8, kind="ExternalInput")
    bqk_d = nc.dram_tensor("bqk", [e, 1], F32, kind="ExternalInput")
    gq_d = nc.dram_tensor("gq", [e, 1], F32, kind="ExternalInput")
    bq_d = nc.dram_tensor("bq", [e, 1], F32, kind="ExternalInput")
    gk_d = nc.dram_tensor("gk", [e, 1], F32, kind="ExternalInput")
    bk_d = nc.dram_tensor("bk", [e, 1], F32, kind="ExternalInput")
    bhv_d = nc.dram_tensor("bhv", [1, hh], BF16, kind="ExternalInput")
    bhgT_d = nc.dram_tensor("bhgT", [P, nctg], F32, kind="ExternalInput")
    xres_d = nc.dram_tensor("xres", [own, dim], F32, kind="ExternalInput")
    out_d = nc.dram_tensor("out", [own, dim], F32, kind="ExternalOutput")

    with TileCtx(nc) as tc, ExitStack() as st:
        constp = st.enter_context(tc.tile_pool(name="const", bufs=1))
        psump = st.enter_context(tc.tile_pool(name="psum", bufs=7,
                                              space="PSUM"))
        psusp = st.enter_context(tc.tile_pool(name="psus", bufs=1,
                                              space="PSUM"))
        dramp = st.enter_context(tc.tile_pool(name="dram", bufs=1,
                                              space="DRAM"))
        mainp = st.enter_context(tc.tile_pool(name="main", bufs=1))

        # v-exchange staging: own v half -> vag_in slab cc; pairwise
        # AllGather per slab; vago[cc] rows = [core0's seq rows, core1's]
        vag_in = dramp.tile([n_cc * seq, IC], FP8, tag="vag_in",
                            name="vag_in")
        vago = [dramp.tile([2 * seq, IC], FP8, tag=f"vago{o}",
                           name=f"vago{o}") for o in range(n_cc)]

        # tiny AllGather to warm the collective stream (first CC op
        # otherwise pays a ~50us cold-start).  Fired AFTER the input burst
        # (see load_warm_ag below) so the stream establishment does not
        # contend with the bandwidth-critical input head; it still
        # completes long before the first real AG's readback is needed.
        warm_in = dramp.tile([1, 64], F32, tag="warm_in", name="warm_in")
        warm_out = dramp.tile([2, 64], F32, tag="warm_out", name="warm_out")
        warm_sb = constp.tile([1, 64], F32, tag="warm_sb")
        nc.vector.memset(warm_sb[:], 0.0)

        scr_sb = constp.tile([P, 2, IC], FP8, tag="scr")
        nc.vector.memset(scr_sb[:], 0.0)

        def pe_warm(n):
            # dummy matmuls on a zeroed scratch tile: keep the PE's HAM
            # activity window busy so (a) the clock is already at 2.4GHz
            # when the first real matmul fires and (b) the input-wait gap
            # after the first k chunk never exceeds the ~3.4us idle window
            # that would re-throttle to K=4/8.  They run entirely inside
            # otherwise-idle head time and their psum is never read.
            for _ in range(n):
                ps = psusp.tile([P, IC], F32, tag="pss", name="pss")
                nc.tensor.matmul(ps[:], scr_sb[:, 0:2, 0:P],
                                 scr_sb[:, 0:2, :], start=True, stop=True,
                                 perf_mode=DR)

        def warm_ag():
            nc.gpsimd.dma_start(warm_in[:], warm_sb[:])
            nc.gpsimd.collective_compute("AllGather", ALU.bypass,
                                         replica_groups=pairs,
                                         ins=[warm_in.opt()],
                                         outs=[warm_out.opt()])

        # ---- constants ----
        wqk_sb = constp.tile([P, nd, e], FP8, tag="wqk")
        nc.scalar.dma_start(wqk_sb[:], wqk_d[:])
        bqk_sb = constp.tile([e, 1], F32, tag="bqk")
        gq_sb = constp.tile([e, 1], F32, tag="gq")
        bq_sb = constp.tile([e, 1], F32, tag="bq")
        gk_sb = constp.tile([e, 1], F32, tag="gk")
        bk_sb = constp.tile([e, 1], F32, tag="bk")
        bhgT_sb = constp.tile([P, nctg], F32, tag="bhgT")
        bhv_sb = constp.tile([1, hh], BF16, tag="bhv")
        ones_sb = constp.tile([1, P], BF16, tag="ones")

        nc.scalar.dma_start(bqk_sb[:], bqk_d[:])

        def load_consts():
            # issued after the whole input stream; everything here is
            # first consumed well past the input head (gate phase or
            # the general-qk path)
            if not fastqk:
                nc.scalar.dma_start(gq_sb[:], gq_d[:])
                nc.scalar.dma_start(bq_sb[:], bq_d[:])
                nc.scalar.dma_start(gk_sb[:], gk_d[:])
                nc.scalar.dma_start(bk_sb[:], bk_d[:])
            nc.scalar.dma_start(bhgT_sb[:], bhgT_d[:])
            if with_bhv:
                nc.scalar.dma_start(bhv_sb[:], bhv_d[:])
                nc.vector.memset(ones_sb[:], 1.0)

        # persistent activations
        qT_sb = mainp.tile([e, own], FP8, tag="qT", name="qT")
        kT_sb = mainp.tile([e, seq], FP8, tag="kT", name="kT")
        v_sb = mainp.tile([P, njt, H], FP8, tag="v", name="v")
        gt_sb = mainp.tile([P, nctg, own], FP8, tag="gt", name="gt")
        at_sb = mainp.tile([P, njt, own], FP8, tag="at", name="at")

        with tc.tile_pool(name="ph1", bufs=1) as ph1p, \
             tc.tile_pool(name="wstream", bufs=1) as wsp:
            # x chunks in SBUF tile layout: [P, chunk, d-tile, col]
            xT_sb = ph1p.tile([P, n_ic, nd, IC], FP8, tag="xT", name="xT")
            xTo_sb = ph1p.tile([P, n_oc, nd, IC], FP8, tag="xTo",
                               name="xTo")
            # Input stream at fabric speed, ordered by first PE use (a
            # single instruction runs ~150GB/s on one queue; split each
            # slab across the three DMA queues): chunk 0, the first wv
            # slab, chunks 1-3, xTo, the second wv slab.
            qs3 = [nc.sync, nc.scalar, nc.gpsimd]
            dspl = [(0, 6), (6, 11), (11, nd)]

            def load_x(sb, dram, ic):
                for q, (d0, d1) in zip(qs3, dspl):
                    q.dma_start(sb[:, ic, d0:d1],
                                dram[ic * P:(ic + 1) * P, d0:d1])

            wv_tiles = {}

            def load_wv(cc, split=False):
                t = wsp.tile([P, nd, IC], FP8, tag="wv", bufs=2, name="wv")
                if split:
                    for q, (d0, d1) in zip(qs3, dspl):
                        q.dma_start(t[:, d0:d1],
                                    whv_d[cc * P:(cc + 1) * P, d0:d1])
                else:
                    nc.sync.dma_start(t[:], whv_d[cc * P:(cc + 1) * P])
                wv_tiles[cc] = t

            wg_tiles = {}

            def load_wg(cg):
                t = wsp.tile([P, nd, CG * P], FP8, tag="wg", bufs=2,
                             name="wg")
                nc.gpsimd.dma_start(t[:], whg_d[cg * P:(cg + 1) * P])
                wg_tiles[cg] = t

            load_x(xT_sb, xT_d, 0)
            load_x(xT_sb, xT_d, 1)
            load_wv(0, split=True)
            for ic in range(2, n_ic):
                load_x(xT_sb, xT_d, ic)
            for ic in range(n_oc):
                load_x(xTo_sb, xTo_d, ic)
            load_wv(1, split=True)
            load_consts()
            warm_ag()
            pe_warm(12)

            # ---- qk projection ----
            # fastqk: one silu ACT per chunk; the whole projection (k for
            # all rows, q for own rows) is interleaved into the v phase,
            # paced to the arrival of its input chunks.  General path:
            # computed up front with the sigmoid + affine rebuild.
            def qk_chunk(rhs_sb, ic, out_sb):
                ps = psump.tile([P, IC], F32, tag="ps", name="ps")
                for kk in range(0, nd, 2):
                    nc.tensor.matmul(ps[:], wqk_sb[:, kk:kk + 2, :],
                                     rhs_sb[:, ic, kk:kk + 2, :],
                                     start=(kk == 0), stop=(kk == nd - 2),
                                     perf_mode=DR)
                nc.scalar.activation(out_sb[:, ic * IC:(ic + 1) * IC], ps[:],
                                     AF.Silu, bias=bqk_sb[:], scale=1.0 / SH)

            if not fastqk:
                with tc.tile_pool(name="qkp", bufs=1) as qkp:
                    def qk_proj(rhs_sb, n_chunks, out_sb, g_sb, b_sb):
                        for ic in range(n_chunks):
                            isl = slice(ic * IC, (ic + 1) * IC)
                            ps = psump.tile([P, IC], F32, tag="ps",
                                            name="ps")
                            for kk in range(0, nd, 2):
                                nc.tensor.matmul(ps[:],
                                                 wqk_sb[:, kk:kk + 2, :],
                                                 rhs_sb[:, ic, kk:kk + 2, :],
                                                 start=(kk == 0),
                                                 stop=(kk == nd - 2),
                                                 perf_mode=DR)
                            sg = qkp.tile([P, IC], F32, tag="sg1", bufs=2,
                                          name="sg")
                            nc.scalar.activation(sg[:], ps[:], AF.Sigmoid,
                                                 bias=bqk_sb[:],
                                                 scale=1.0 / SH)
                            u = qkp.tile([P, IC], F32, tag="u1", bufs=1,
                                         name="u")
                            nc.vector.tensor_scalar(u[:], ps[:], 1.0 / SH,
                                                    bqk_sb[:], ALU.mult,
                                                    ALU.add)
                            qkf = qkp.tile([P, IC], F32, tag="qkf", bufs=1,
                                           name="qkf")
                            nc.vector.tensor_tensor(qkf[:], u[:], sg[:],
                                                    ALU.mult)
                            nc.vector.tensor_scalar(out_sb[:, isl], qkf[:],
                                                    g_sb[:], b_sb[:],
                                                    ALU.mult, ALU.add)

                    qk_proj(xT_sb, n_ic, kT_sb, gk_sb, bk_sb)
                    qk_proj(xTo_sb, n_oc, qT_sb, gq_sb, bq_sb)

            # ---- hidden (v part) with qk / scores interleaved ----
            # The k and q projection chunks (fastqk) and the 32 scores
            # matmuls (K=128, no DoubleRow) ride inside the v stream, at
            # most one extra per v group, gated on when their inputs
            # arrive; scores go on a private 2-bank psum ring drained by
            # the otherwise-idle DVE (relu^2 as mult+max then square).
            # This keeps the PE fully fed while the input stream (which
            # delivers ~1MB per 3.6us against a PE appetite of 1MB per
            # 1.7us of pure-v work) catches up.
            if fastqk:
                extra = ([('k', 0), ('d',), ('k', 1)]
                         + [('k', ic) for ic in range(2, n_ic)]
                         + [('q', ic) for ic in range(n_oc)]
                         + [('s', ic, jt) for ic in range(n_oc)
                            for jt in range(njt)])
                thr = [0, 0, 0, 8, 11, 13, 15] + [17] * (n_oc * njt)
            else:
                extra = [('s', ic, jt) for ic in range(n_oc)
                         for jt in range(njt)]
                thr = [0] * (n_oc * njt)
            ei = 0

            def emit_extra(g):
                nonlocal ei
                # the qk chunks may fire several per slot (they are what
                # the PE chews on while the input stream catches up); the
                # scores stay one-per-group so the DVE keeps pace
                while ei < len(extra) and g >= thr[ei]:
                    it = extra[ei]
                    ei += 1
                    if it[0] == 'd':
                        pe_warm(8)
                        continue
                    if it[0] == 'k':
                        qk_chunk(xT_sb, it[1], kT_sb)
                        continue
                    if it[0] == 'q':
                        qk_chunk(xTo_sb, it[1], qT_sb)
                        continue
                    _, ic, jt = it
                    isl = slice(ic * IC, (ic + 1) * IC)
                    ps = psusp.tile([P, IC], F32, tag="pss", name="pss")
                    nc.tensor.matmul(ps[:], kT_sb[:, jt * P:(jt + 1) * P],
                                     qT_sb[:, isl], start=True, stop=True)
                    rs = wsp.tile([P, IC], BF16, tag="rs", bufs=1,
                                  name="rs")
                    nc.vector.tensor_scalar(rs[:], ps[:], rst, 0.0,
                                            ALU.mult, ALU.max)
                    nc.vector.tensor_tensor(at_sb[:, jt, isl], rs[:], rs[:],
                                            ALU.mult)
                    break

            # v goes to DRAM (own half); AllGather fires per column slab.
            # vst accumulates 4 j-tiles so each write is one instruction.
            vg = 0
            VJ = 4
            for cc in range(n_cc):
                csl = slice(cc * IC, (cc + 1) * IC)
                if cc + 2 < n_cc:
                    load_wv(cc + 2)
                wv = wv_tiles.pop(cc)
                for jt in range(njt):
                    emit_extra(vg)
                    vg += 1
                    ps = psump.tile([P, IC], F32, tag="ps", name="ps")
                    for kk in range(0, nd, 2):
                        nc.tensor.matmul(
                            ps[:],
                            xT_sb[:, jt // n_it, kk:kk + 2,
                                  (jt % n_it) * P:(jt % n_it + 1) * P],
                            wv[:, kk:kk + 2, :],
                            start=(kk == 0),
                            stop=(not with_bhv and kk == nd - 2),
                            perf_mode=DR)
                    if with_bhv:
                        # bhv host-scaled by 32 to match the psum scale
                        nc.tensor.matmul(ps[:], ones_sb[:], bhv_sb[:, csl],
                                         start=False, stop=True,
                                         skip_group_check=True)
                    if jt % VJ == 0:
                        vst = wsp.tile([P, VJ, IC], FP8, bufs=2, tag="vst",
                                       name="vst")
                    nc.scalar.activation(vst[:, jt % VJ, :], ps[:], AF.Silu,
                                         scale=1.0 / SH)
                    if jt % VJ == VJ - 1:
                        # one instruction per VJ j-tiles; on scalar (right
                        # behind the ACTs that produce it) so the gpsimd
                        # queue carries only the AllGather triggers
                        nc.scalar.dma_start(
                            vag_in[cc * seq + (jt - VJ + 1) * P:
                                   cc * seq + (jt + 1) * P,
                                   :].rearrange("(j p) c -> p j c", p=P),
                            vst[:])
                if cc == n_cc - 2:
                    # prefetch the first gate weight groups now, before the
                    # late AG triggers occupy the gpsimd queue
                    load_wg(0)
                if cc == n_cc - 1:
                    load_wg(1)
                nc.gpsimd.collective_compute(
                    "AllGather", ALU.bypass, replica_groups=pairs,
                    ins=[vag_in[cc * seq:(cc + 1) * seq, :].opt()],
                    outs=[vago[cc].opt()])
            while ei < len(extra):
                emit_extra(10 ** 9)

            # ---- hidden (gate part) ----
            # v readbacks are just-in-time: attn@v consumes v one column
            # slab at a time, so slab s (= half g, slab cc, covering og
            # c-tiles 4s..4s+3) is read back two slabs ahead of use --
            # the first two under the last gate groups, the rest inside
            # the attn@v loop.  By then every AllGather has long
            # completed, so the readback DMAs never block a queue on an
            # in-flight collective.  One instruction per slab.
            def v_readback_slab(s):
                g, cc = divmod(s, n_cc)
                nc.sync.dma_start(
                    v_sb[:, :, g * hh + cc * IC:g * hh + (cc + 1) * IC],
                    vago[cc][g * seq:(g + 1) * seq, :].rearrange(
                        "(j p) c -> p j c", p=P))

            for cg in range(n_cg):
                if cg + 2 < n_cg:
                    load_wg(cg + 2)
                wg = wg_tiles.pop(cg)
                if cg == n_cg - 2:
                    v_readback_slab(0)
                if cg == n_cg - 1:
                    v_readback_slab(1)
                for cl in range(CG):
                    ct = cg * CG + cl
                    for ic in range(n_oc):
                        isl = slice(ic * IC, (ic + 1) * IC)
                        ps = psump.tile([P, IC], F32, tag="ps", name="ps")
                        for kk in range(0, nd, 2):
                            nc.tensor.matmul(ps[:],
                                             wg[:, kk:kk + 2,
                                                cl * P:(cl + 1) * P],
                                             xTo_sb[:, ic, kk:kk + 2, :],
                                             start=(kk == 0),
                                             stop=(kk == nd - 2),
                                             perf_mode=DR)
                        # gate = silu(psum/SH + b), fp8 at true scale
                        nc.scalar.activation(gt_sb[:, ct, isl], ps[:],
                                             AF.Silu,
                                             bias=bhgT_sb[:, ct:ct + 1],
                                             scale=1.0 / SH)

        # ---- attention output + final projection (own rows only) ----
        with tc.tile_pool(name="ph2", bufs=1) as ph2p:
            og_sb = [ph2p.tile([P, nctg, IC], FP8, tag=f"og{i}",
                               name=f"og{i}") for i in range(n_oc)]
            # ogT[all c, chunk] = (v^T @ attnT) * gateT, both chunks first
            # (both og buffers stay live so the out-projection can then run
            # dc-outer across chunks, loading each Wout column-block ONCE)
            n_slab = 2 * n_cc            # v column slabs
            ctps = nctg // n_slab        # og c-tiles per slab
            for ic in range(n_oc):
                isl = slice(ic * IC, (ic + 1) * IC)
                og = og_sb[ic]
                for ct in range(nctg):
                    if ic == 0 and ct % ctps == 0 and ct // ctps + 2 < n_slab:
                        v_readback_slab(ct // ctps + 2)
                    ps = psump.tile([P, IC], F32, tag="ps", name="ps")
                    for kk in range(0, njt, 2):
                        nc.tensor.matmul(ps[:],
                                         v_sb[:, kk:kk + 2, ct * P:(ct + 1) * P],
                                         at_sb[:, kk:kk + 2, isl],
                                         start=(kk == 0), stop=(kk == njt - 2),
                                         perf_mode=DR)
                    nc.vector.tensor_tensor(og[:, ct, :], ps[:],
                                            gt_sb[:, ct, isl], ALU.mult)
            # final rows: out[own rows, :] = POSC ogT^T Wout + xres
            for dc in range(n_dc):
                wo = ph2p.tile([P, nctg, DC], FP8, tag="wo", bufs=2,
                              name="wo")
                nc.gpsimd.dma_start(wo[:], wout_d[dc * P:(dc + 1) * P])
                # all residual rows for this column block load up front
                # (split over two queues) so the write-out chain never
                # waits on them -- the late-xr wait used to stretch the
                # end-of-kernel drain by ~8us
                xrs = []
                for t in range(n_oc * n_it):
                    xr = ph2p.tile([P, DC], F32, tag="xr",
                                   bufs=n_oc * n_it - 2, name="xr")
                    q = nc.sync if t % 2 else nc.scalar
                    q.dma_start(xr[:],
                                xres_d[t * P:(t + 1) * P,
                                       dc * DC:(dc + 1) * DC])
                    xrs.append(xr)
                for ic in range(n_oc):
                    for it in range(n_it):
                        orow = ic * IC + it * P
                        xr = xrs[ic * n_it + it]
                        ps = psump.tile([P, DC], F32, tag="ps", name="ps")
                        for kk in range(0, nctg, 2):
                            nc.tensor.matmul(ps[:],
                                             og_sb[ic][:, kk:kk + 2,
                                                       it * P:(it + 1) * P],
                                             wo[:, kk:kk + 2, :],
                                             start=(kk == 0),
                                             stop=(kk == nctg - 2),
                                             perf_mode=DR)
                        po = ph2p.tile([P, DC], F32, tag="po", bufs=2,
                                       name="po")
                        fo = ph2p.tile([P, DC], F32, tag="fo", bufs=2,
                                       name="fo")
                        last = (dc == n_dc - 1 and ic == n_oc - 1
                                and it == n_it - 1)
                        # the very last tile drains in narrow strips so the
                        # end-of-kernel ACT->DVE->DMA chain is short
                        nst = 4 if last else 1
                        sw = DC // nst
                        for st_ in range(nst):
                            ssl = slice(st_ * sw, (st_ + 1) * sw)
                            nc.scalar.mul(po[:, ssl], ps[:, ssl], POSC)
                            nc.vector.tensor_tensor(fo[:, ssl], xr[:, ssl],
                                                    po[:, ssl], ALU.add)
                            wq = (nc.scalar
                                  if (ic * n_it + it + st_) % 2 else nc.sync)
                            wq.dma_start(
                                out_d[orow:orow + P,
                                      dc * DC + st_ * sw:
                                      dc * DC + (st_ + 1) * sw],
                                fo[:, ssl])

    nc.compile()
    return nc


def TileCtx(nc):
    return tile.TileContext(nc)


def own_rows(seq, h, IC_=None):
    """Rows owned by pair-member h: the contiguous h-th half."""
    return np.arange(h * (seq // 2), (h + 1) * (seq // 2))


def _to_fp8(a):
    return np.clip(a, -224.0, 224.0).astype(ml_dtypes.float8_e4m3)


def make_in_maps(x, W_hidden, b_hidden, W_qk, b_qk, gamma_q, beta_q,
                 gamma_k, beta_k, W_out, b_out, n_cores=8):
    """Host-side sharding/layout prep.  Returns per-core input dicts."""
    B, seq, dim = x.shape
    H2 = W_hidden.shape[1]
    H = H2 // 2
    hh = H // 2  # per-core v-half width
    nctg = H // P
    in_maps = []
    xT_cache = {}
    whg8 = _tile_pack(_to_fp8(W_hidden[:, H:] * SH), P, nctg // CG, CG * P)
    wout8 = _tile_pack(_to_fp8(W_out * SO), P, dim // DC, DC)
    wqk8 = _to_fp8(np.ascontiguousarray(
        np.concatenate(np.split(W_qk * SH, dim // P, axis=0), axis=1)))
    bhgT = np.ascontiguousarray(
        b_hidden[H:].reshape(nctg, P).T).astype(np.float32)
    whv8 = {}
    for core in range(n_cores):
        b, h = core // 2, core % 2
        if b not in xT_cache:
            xT8 = _to_fp8(np.ascontiguousarray(x[b].T))
            rows = own_rows(seq, h)
            xT_cache[b] = (
                _tile_pack(xT8, P, seq // IC, IC),
                xT8,
            )
        rows = own_rows(seq, h)
        xres = (x[b][rows].astype(np.float32)
                + b_out.astype(np.float32)[None, :])
        cs = slice(h * hh, (h + 1) * hh)
        if h not in whv8:
            whv8[h] = _tile_pack(_to_fp8(W_hidden[:, cs] * SH),
                                 P, hh // IC, IC)
        in_maps.append({
            "xT": xT_cache[b][0],
            "xTo": _tile_pack(
                np.ascontiguousarray(xT_cache[b][1][:, rows]),
                P, (seq // 2) // IC, IC),
            "whv": whv8[h],
            "whg": whg8,
            "wqk": wqk8,
            "wout": wout8,
            "bqk": b_qk.reshape(-1, 1).astype(np.float32),
            "gq": gamma_q.reshape(-1, 1).astype(np.float32),
            "bq": beta_q.reshape(-1, 1).astype(np.float32),
            "gk": gamma_k.reshape(-1, 1).astype(np.float32),
            "bk": beta_k.reshape(-1, 1).astype(np.float32),
            "bhv": (b_hidden[cs] * SH).reshape(1, -1).astype(
                ml_dtypes.bfloat16),
            "bhgT": bhgT,
            "xres": xres,
        })
    return in_maps


_NC_CACHE = {}


def _get_nc(seq, dim, hh, n_cores, with_bhv=True, fastqk=True):
    key = (seq, dim, hh, n_cores, with_bhv, fastqk)
    if key not in _NC_CACHE:
        _NC_CACHE[key] = build_gau_nc(seq=seq, dim=dim, hh=hh,
                                      n_cores=n_cores, with_bhv=with_bhv,
                                      fastqk=fastqk)
    return _NC_CACHE[key]


def _is_fastqk(gamma_q, beta_q, gamma_k, beta_k):
    return bool(np.all(gamma_q == 1.0) and np.all(beta_q == 0.0)
                and np.all(gamma_k == 1.0) and np.all(beta_k == 0.0))


def kernel(x, W_hidden, b_hidden, W_qk, b_qk, gamma_q, beta_q, gamma_k,
           beta_k, W_out, b_out):
    x = np.asarray(x)
    B, seq, dim = x.shape
    hh = W_hidden.shape[1] // 4
    n_cores = 2 * B
    with_bhv = bool(np.any(np.asarray(b_hidden)[: 2 * hh] != 0))
    fastqk = _is_fastqk(np.asarray(gamma_q), np.asarray(beta_q),
                        np.asarray(gamma_k), np.asarray(beta_k))
    nc = _get_nc(seq, dim, hh, n_cores, with_bhv=with_bhv, fastqk=fastqk)
    in_maps = make_in_maps(x, np.asarray(W_hidden), np.asarray(b_hidden),
                           np.asarray(W_qk), np.asarray(b_qk),
                           np.asarray(gamma_q), np.asarray(beta_q),
                           np.asarray(gamma_k), np.asarray(beta_k),
                           np.asarray(W_out), np.asarray(b_out),
                           n_cores=n_cores)
    res = run_bass_kernel_spmd(nc, in_maps, core_ids=list(range(n_cores)))
    out = np.empty((B, seq, dim), np.float32)
    for b in range(B):
        for h in range(2):
            out[b, own_rows(seq, h)] = res.results[2 * b + h]["out"]
    return out


# revision 41
# speedup vs baseline: 1.0225x; 1.0080x over previous
"""GAU (Gated Attention Unit, relu^2 attention) Trainium2 Bass kernel, FP8.

Problem shapes: x [4, 2048, 2048] f32; W_hidden [2048, 8192]; W_qk [2048, 128];
W_out [4096, 2048]; out = GAU(x) + x.

Sharding (8 cores): core = 2*batch + h, h in {0,1}.  Each pair of cores
handles one batch.  Phase 1 splits the v-half of the hidden dim (each core
computes v for its 2048 columns); the halves are exchanged with pairwise
AllGathers (DRAM-staged, one per 512-column slab, each fired the moment its
slab completes during the v phase).  Everything downstream is split by
sequence position: each core computes the gate, the attention rows, the
gated output and the final projection ONLY for its own contiguous 1024
rows, using the full 4096-wide hidden dim -- so there is no end-of-kernel
ReduceScatter and no cross-core reduction.  The qk projection is computed
once for all positions (for k) plus once for own positions (for q, via the
host-sliced xTown input -- SPMD code cannot index by core id).

Schedule (single uninterrupted PE stream, phases in program order):
  k proj -> hidden-v (with the q projection and the 32 attention-scores
  matmuls interleaved into the stream, scores drained by the otherwise-idle
  DVE) -> gate -> attn@v (v readbacks ride just-in-time, one column slab
  ahead of consumption) -> out projection.

DMA strategy: x / xTo / W_hidden / W_out are repacked on the host into the
exact SBUF tile layout ([128 partition rows, ...free]) so that every load
is ONE dma instruction per tile with 8KB-contiguous packets -- the DMA
issue rate (~620ns per instruction) and the 512B-packet rate ceilings both
disappear, and the input head runs at fabric bandwidth.  v-exchange
readbacks use a (j p) c -> p j c rearrange to land one whole slab per
instruction.

All large matmuls run in fp8 e4m3 with perf_mode=DoubleRow (K=256 per
instruction); the small attention-scores matmul (K=128) is plain fp8.
fp32 PSUM accumulation throughout.  The GAU branch is ~3e-3 of the output
norm (the residual dominates), so ~10% fp8 error on the branch is ~3e-4
end-to-end.

fp8 range management (TRN e4m3 max +-240; >240 converts to Inf, subnormal
floor 2^-9).  Host pre-scales W_hidden and W_qk by 32 and W_out by 64 so
their rms is ~1.  Carried scales, verified against the actual seed-0 data:

  xT fp8           = x^T              (max 5.4)
  v_fp8            = v                (true scale, max 4.9)
  gate_fp8         = gate             (true scale, max 5.3)
  qT/kT fp8        true scale         (max 4.8)
  at_fp8           = SA attn          (SA=seq=2048: max 3.7; relu stage
                                       scale sqrt(SA)/seq)
  og_fp8 = (SA attn@v) * gate = SA og (max 122)
  out psum = og_fp8^T (64 Wout) = SA*64 branch -> out = xres + psum/(SA*64)
"""

import math
import numpy as np
import ml_dtypes
from contextlib import ExitStack

import concourse.bass as bass
import concourse.bacc as bacc
import concourse.mybir as mybir
import concourse.tile as tile
from concourse.bass_utils import run_bass_kernel_spmd

BF16 = mybir.dt.bfloat16
F32 = mybir.dt.float32
FP8 = mybir.dt.float8e4
DR = mybir.MatmulPerfMode.DoubleRow
AF = mybir.ActivationFunctionType
ALU = mybir.AluOpType
P = 128

SH = 32.0          # host scale on W_hidden / W_qk
SO = 64.0          # host scale on W_out
IC = 512           # moving-dim chunk (and v slab width)
DC = 512           # out-projection column block
CG = 4             # gate c-tiles per weight group


def _tile_pack(w, P_, nchunk, width):
    """[K, nchunk*width] row-major -> [nchunk*P_, K//P_, width] so that
    chunk i is a [P_, K//P_, width] block matching the SBUF tile layout
    (partition-major, 8KB-contiguous per partition row)."""
    K = w.shape[0]
    nd = K // P_
    return np.ascontiguousarray(
        w.reshape(nd, P_, nchunk, width).transpose(2, 1, 0, 3)
        .reshape(nchunk * P_, nd, width))


def build_gau_nc(seq=2048, dim=2048, hh=2048, n_cores=8,
                 with_bhv=True, fastqk=True):
    """Build the SPMD program.  hh = per-core v-half width (H/2).

    fastqk: gamma_q == gamma_k == 1 and beta_q == beta_k == 0, so q and k
    are both just silu(x W_qk + b_qk) and the projection is one ACT op.
    """
    e = P  # qk dim
    H = 2 * hh          # full hidden width
    own = seq // 2      # own sequence rows per core
    nd = dim // P       # d-tiles (contraction tiles for x)
    njt = seq // P      # seq tiles (j)
    n_ic = seq // IC           # chunks of the full sequence
    n_oc = own // IC           # chunks of the own half
    nctg = H // P       # full-hidden c-tiles (gate/og/wout)
    n_dc = dim // DC
    n_it = IC // P
    n_cc = hh // IC     # v-half column slabs (one AllGather each)
    n_cg = nctg // CG   # gate weight groups
    SA = float(seq)     # fp8 scale of the attention matrix
    rst = math.sqrt(SA) / seq
    POSC = 1.0 / (SA * SO)     # final psum descale
    pairs = [[2 * g, 2 * g + 1] for g in range(n_cores // 2)]

    nc = bacc.Bacc("TRN2", target_bir_lowering=False, debug=False,
                   num_devices=n_cores)

    # all big operands host-repacked to SBUF tile layout (see _tile_pack)
    xT_d = nc.dram_tensor("xT", [n_ic * P, nd, IC], FP8,
                          kind="ExternalInput")
    xTo_d = nc.dram_tensor("xTo", [n_oc * P, nd, IC], FP8,
                           kind="ExternalInput")
    whv_d = nc.dram_tensor("whv", [n_cc * P, nd, IC], FP8,
                           kind="ExternalInput")
    whg_d = nc.dram_tensor("whg", [n_cg * P, nd, CG * P], FP8,
                           kind="ExternalInput")
    wout_d = nc.dram_tensor("wout", [n_dc * P, nctg, DC], FP8,
                            kind="ExternalInput")
    wqk_d = nc.dram_tensor("wqk", [P, nd * e], FP8, kind="ExternalInput")
    bqk_d = nc.dram_tensor("bqk", [e, 1], F32, kind="ExternalInput")
    gq_d = nc.dram_tensor("gq", [e, 1], F32, kind="ExternalInput")
    bq_d = nc.dram_tensor("bq", [e, 1], F32, kind="ExternalInput")
    gk_d = nc.dram_tensor("gk", [e, 1], F32, kind="ExternalInput")
    bk_d = nc.dram_tensor("bk", [e, 1], F32, kind="ExternalInput")
    bhv_d = nc.dram_tensor("bhv", [1, hh], BF16, kind="ExternalInput")
    bhgT_d = nc.dram_tensor("bhgT", [P, nctg], F32, kind="ExternalInput")
    xres_d = nc.dram_tensor("xres", [own, dim], F32, kind="ExternalInput")
    out_d = nc.dram_tensor("out", [own, dim], F32, kind="ExternalOutput")

    with TileCtx(nc) as tc, ExitStack() as st:
        constp = st.enter_context(tc.tile_pool(name="const", bufs=1))
        psump = st.enter_context(tc.tile_pool(name="psum", bufs=7,
                                              space="PSUM"))
        psusp = st.enter_context(tc.tile_pool(name="psus", bufs=1,
                                              space="PSUM"))
        dramp = st.enter_context(tc.tile_pool(name="dram", bufs=1,
                                              space="DRAM"))
        mainp = st.enter_context(tc.tile_pool(name="main", bufs=1))

        # v-exchange staging: own v half -> vag_in slab cc; pairwise
        # AllGather per slab; vago[cc] rows = [core0's seq rows, core1's]
        vag_in = dramp.tile([n_cc * seq, IC], FP8, tag="vag_in",
                            name="vag_in")
        vago = [dramp.tile([2 * seq, IC], FP8, tag=f"vago{o}",
                           name=f"vago{o}") for o in range(n_cc)]

        # tiny AllGather to warm the collective stream (first CC op
        # otherwise pays a ~50us cold-start).  Fired AFTER the input burst
        # (see load_warm_ag below) so the stream establishment does not
        # contend with the bandwidth-critical input head; it still
        # completes long before the first real AG's readback is needed.
        warm_in = dramp.tile([1, 64], F32, tag="warm_in", name="warm_in")
        warm_out = dramp.tile([2, 64], F32, tag="warm_out", name="warm_out")
        warm_sb = constp.tile([1, 64], F32, tag="warm_sb")
        nc.vector.memset(warm_sb[:], 0.0)

        scr_sb = constp.tile([P, 2, IC], FP8, tag="scr")
        nc.vector.memset(scr_sb[:], 0.0)

        def pe_warm(n):
            # dummy matmuls on a zeroed scratch tile: keep the PE's HAM
            # activity window busy so (a) the clock is already at 2.4GHz
            # when the first real matmul fires and (b) the input-wait gap
            # after the first k chunk never exceeds the ~3.4us idle window
            # that would re-throttle to K=4/8.  One accumulation group so
            # the matmuls run back-to-back (no per-matmul psum-slot
            # serialization for the scheduler to interleave real work
            # into); the psum is never read.
            ps = psusp.tile([P, IC], F32, tag="pss", name="pss")
            for i in range(n):
                nc.tensor.matmul(ps[:], scr_sb[:, 0:2, 0:P],
                                 scr_sb[:, 0:2, :], start=(i == 0),
                                 stop=(i == n - 1), perf_mode=DR)

        def warm_ag():
            nc.gpsimd.dma_start(warm_in[:], warm_sb[:])
            nc.gpsimd.collective_compute("AllGather", ALU.bypass,
                                         replica_groups=pairs,
                                         ins=[warm_in.opt()],
                                         outs=[warm_out.opt()])

        # ---- constants ----
        wqk_sb = constp.tile([P, nd, e], FP8, tag="wqk")
        nc.scalar.dma_start(wqk_sb[:], wqk_d[:])
        bqk_sb = constp.tile([e, 1], F32, tag="bqk")
        gq_sb = constp.tile([e, 1], F32, tag="gq")
        bq_sb = constp.tile([e, 1], F32, tag="bq")
        gk_sb = constp.tile([e, 1], F32, tag="gk")
        bk_sb = constp.tile([e, 1], F32, tag="bk")
        bhgT_sb = constp.tile([P, nctg], F32, tag="bhgT")
        bhv_sb = constp.tile([1, hh], BF16, tag="bhv")
        ones_sb = constp.tile([1, P], BF16, tag="ones")

        nc.scalar.dma_start(bqk_sb[:], bqk_d[:])

        def load_consts():
            # issued after the whole input stream; everything here is
            # first consumed well past the input head (gate phase or
            # the general-qk path)
            if not fastqk:
                nc.scalar.dma_start(gq_sb[:], gq_d[:])
                nc.scalar.dma_start(bq_sb[:], bq_d[:])
                nc.scalar.dma_start(gk_sb[:], gk_d[:])
                nc.scalar.dma_start(bk_sb[:], bk_d[:])
            nc.scalar.dma_start(bhgT_sb[:], bhgT_d[:])
            if with_bhv:
                nc.scalar.dma_start(bhv_sb[:], bhv_d[:])
                nc.vector.memset(ones_sb[:], 1.0)

        # persistent activations
        qT_sb = mainp.tile([e, own], FP8, tag="qT", name="qT")
        kT_sb = mainp.tile([e, seq], FP8, tag="kT", name="kT")
        v_sb = mainp.tile([P, njt, H], FP8, tag="v", name="v")
        gt_sb = mainp.tile([P, nctg, own], FP8, tag="gt", name="gt")
        at_sb = mainp.tile([P, njt, own], FP8, tag="at", name="at")

        with tc.tile_pool(name="ph1", bufs=1) as ph1p, \
             tc.tile_pool(name="wstream", bufs=1) as wsp:
            # x chunks in SBUF tile layout: [P, chunk, d-tile, col]
            xT_sb = ph1p.tile([P, n_ic, nd, IC], FP8, tag="xT", name="xT")
            xTo_sb = ph1p.tile([P, n_oc, nd, IC], FP8, tag="xTo",
                               name="xTo")
            # Input stream at fabric speed, ordered by first PE use (a
            # single instruction runs ~150GB/s on one queue; split each
            # slab across the three DMA queues): chunk 0, the first wv
            # slab, chunks 1-3, xTo, the second wv slab.
            qs3 = [nc.sync, nc.scalar, nc.gpsimd]
            dspl = [(0, 6), (6, 11), (11, nd)]

            def load_x(sb, dram, ic):
                for q, (d0, d1) in zip(qs3, dspl):
                    q.dma_start(sb[:, ic, d0:d1],
                                dram[ic * P:(ic + 1) * P, d0:d1])

            wv_tiles = {}

            def load_wv(cc, split=False):
                t = wsp.tile([P, nd, IC], FP8, tag="wv", bufs=2, name="wv")
                if split:
                    for q, (d0, d1) in zip(qs3, dspl):
                        q.dma_start(t[:, d0:d1],
                                    whv_d[cc * P:(cc + 1) * P, d0:d1])
                else:
                    nc.sync.dma_start(t[:], whv_d[cc * P:(cc + 1) * P])
                wv_tiles[cc] = t

            wg_tiles = {}

            def load_wg(cg):
                t = wsp.tile([P, nd, CG * P], FP8, tag="wg", bufs=2,
                             name="wg")
                nc.gpsimd.dma_start(t[:], whg_d[cg * P:(cg + 1) * P])
                wg_tiles[cg] = t

            load_x(xT_sb, xT_d, 0)
            load_x(xT_sb, xT_d, 1)
            load_wv(0, split=True)
            for ic in range(2, n_ic):
                load_x(xT_sb, xT_d, ic)
            for ic in range(n_oc):
                load_x(xTo_sb, xTo_d, ic)
            load_wv(1, split=True)
            load_consts()
            warm_ag()
            pe_warm(12)

            # ---- qk projection ----
            # fastqk: one silu ACT per chunk; the whole projection (k for
            # all rows, q for own rows) is interleaved into the v phase,
            # paced to the arrival of its input chunks.  General path:
            # computed up front with the sigmoid + affine rebuild.
            def qk_chunk(rhs_sb, ic, out_sb):
                ps = psump.tile([P, IC], F32, tag="ps", name="ps")
                for kk in range(0, nd, 2):
                    nc.tensor.matmul(ps[:], wqk_sb[:, kk:kk + 2, :],
                                     rhs_sb[:, ic, kk:kk + 2, :],
                                     start=(kk == 0), stop=(kk == nd - 2),
                                     perf_mode=DR)
                nc.scalar.activation(out_sb[:, ic * IC:(ic + 1) * IC], ps[:],
                                     AF.Silu, bias=bqk_sb[:], scale=1.0 / SH)

            if not fastqk:
                with tc.tile_pool(name="qkp", bufs=1) as qkp:
                    def qk_proj(rhs_sb, n_chunks, out_sb, g_sb, b_sb):
                        for ic in range(n_chunks):
                            isl = slice(ic * IC, (ic + 1) * IC)
                            ps = psump.tile([P, IC], F32, tag="ps",
                                            name="ps")
                            for kk in range(0, nd, 2):
                                nc.tensor.matmul(ps[:],
                                                 wqk_sb[:, kk:kk + 2, :],
                                                 rhs_sb[:, ic, kk:kk + 2, :],
                                                 start=(kk == 0),
                                                 stop=(kk == nd - 2),
                                                 perf_mode=DR)
                            sg = qkp.tile([P, IC], F32, tag="sg1", bufs=2,
                                          name="sg")
                            nc.scalar.activation(sg[:], ps[:], AF.Sigmoid,
                                                 bias=bqk_sb[:],
                                                 scale=1.0 / SH)
                            u = qkp.tile([P, IC], F32, tag="u1", bufs=1,
                                         name="u")
                            nc.vector.tensor_scalar(u[:], ps[:], 1.0 / SH,
                                                    bqk_sb[:], ALU.mult,
                                                    ALU.add)
                            qkf = qkp.tile([P, IC], F32, tag="qkf", bufs=1,
                                           name="qkf")
                            nc.vector.tensor_tensor(qkf[:], u[:], sg[:],
                                                    ALU.mult)
                            nc.vector.tensor_scalar(out_sb[:, isl], qkf[:],
                                                    g_sb[:], b_sb[:],
                                                    ALU.mult, ALU.add)

                    qk_proj(xT_sb, n_ic, kT_sb, gk_sb, bk_sb)
                    qk_proj(xTo_sb, n_oc, qT_sb, gq_sb, bq_sb)

            # ---- hidden (v part) with qk / scores interleaved ----
            # The k and q projection chunks (fastqk) and the 32 scores
            # matmuls (K=128, no DoubleRow) ride inside the v stream, at
            # most one extra per v group, gated on when their inputs
            # arrive; scores go on a private 2-bank psum ring drained by
            # the otherwise-idle DVE (relu^2 as mult+max then square).
            # This keeps the PE fully fed while the input stream (which
            # delivers ~1MB per 3.6us against a PE appetite of 1MB per
            # 1.7us of pure-v work) catches up.
            if fastqk:
                extra = ([('k', 0), ('d',), ('k', 1)]
                         + [('k', ic) for ic in range(2, n_ic)]
                         + [('q', ic) for ic in range(n_oc)]
                         + [('s', ic, jt) for ic in range(n_oc)
                            for jt in range(njt)])
                thr = [0, 0, 0, 8, 11, 13, 15] + [17] * (n_oc * njt)
            else:
                extra = [('s', ic, jt) for ic in range(n_oc)
                         for jt in range(njt)]
                thr = [0] * (n_oc * njt)
            ei = 0

            def emit_extra(g):
                nonlocal ei
                # the qk chunks may fire several per slot (they are what
                # the PE chews on while the input stream catches up); the
                # scores stay one-per-group so the DVE keeps pace
                while ei < len(extra) and g >= thr[ei]:
                    it = extra[ei]
                    ei += 1
                    if it[0] == 'd':
                        pe_warm(8)
                        continue
                    if it[0] == 'k':
                        qk_chunk(xT_sb, it[1], kT_sb)
                        continue
                    if it[0] == 'q':
                        qk_chunk(xTo_sb, it[1], qT_sb)
                        continue
                    _, ic, jt = it
                    isl = slice(ic * IC, (ic + 1) * IC)
                    ps = psusp.tile([P, IC], F32, tag="pss", name="pss")
                    nc.tensor.matmul(ps[:], kT_sb[:, jt * P:(jt + 1) * P],
                                     qT_sb[:, isl], start=True, stop=True)
                    rs = wsp.tile([P, IC], BF16, tag="rs", bufs=1,
                                  name="rs")
                    nc.vector.tensor_scalar(rs[:], ps[:], rst, 0.0,
                                            ALU.mult, ALU.max)
                    nc.vector.tensor_tensor(at_sb[:, jt, isl], rs[:], rs[:],
                                            ALU.mult)
                    break

            # v goes to DRAM (own half); AllGather fires per column slab.
            # vst accumulates 4 j-tiles so each write is one instruction.
            vg = 0
            VJ = 4
            for cc in range(n_cc):
                csl = slice(cc * IC, (cc + 1) * IC)
                if cc + 2 < n_cc:
                    load_wv(cc + 2)
                wv = wv_tiles.pop(cc)
                for jt in range(njt):
                    emit_extra(vg)
                    vg += 1
                    ps = psump.tile([P, IC], F32, tag="ps", name="ps")
                    for kk in range(0, nd, 2):
                        nc.tensor.matmul(
                            ps[:],
                            xT_sb[:, jt // n_it, kk:kk + 2,
                                  (jt % n_it) * P:(jt % n_it + 1) * P],
                            wv[:, kk:kk + 2, :],
                            start=(kk == 0),
                            stop=(not with_bhv and kk == nd - 2),
                            perf_mode=DR)
                    if with_bhv:
                        # bhv host-scaled by 32 to match the psum scale
                        nc.tensor.matmul(ps[:], ones_sb[:], bhv_sb[:, csl],
                                         start=False, stop=True,
                                         skip_group_check=True)
                    if jt % VJ == 0:
                        vst = wsp.tile([P, VJ, IC], FP8, bufs=2, tag="vst",
                                       name="vst")
                    nc.scalar.activation(vst[:, jt % VJ, :], ps[:], AF.Silu,
                                         scale=1.0 / SH)
                    if jt % VJ == VJ - 1:
                        # one instruction per VJ j-tiles; on scalar (right
                        # behind the ACTs that produce it) so the gpsimd
                        # queue carries only the AllGather triggers
                        nc.scalar.dma_start(
                            vag_in[cc * seq + (jt - VJ + 1) * P:
                                   cc * seq + (jt + 1) * P,
                                   :].rearrange("(j p) c -> p j c", p=P),
                            vst[:])
                if cc == n_cc - 2:
                    # prefetch the first gate weight groups now, before the
                    # late AG triggers occupy the gpsimd queue
                    load_wg(0)
                if cc == n_cc - 1:
                    load_wg(1)
                nc.gpsimd.collective_compute(
                    "AllGather", ALU.bypass, replica_groups=pairs,
                    ins=[vag_in[cc * seq:(cc + 1) * seq, :].opt()],
                    outs=[vago[cc].opt()])
            while ei < len(extra):
                emit_extra(10 ** 9)

            # ---- hidden (gate part) ----
            # v readbacks are just-in-time: attn@v consumes v one column
            # slab at a time, so slab s (= half g, slab cc, covering og
            # c-tiles 4s..4s+3) is read back two slabs ahead of use --
            # the first two under the last gate groups, the rest inside
            # the attn@v loop.  By then every AllGather has long
            # completed, so the readback DMAs never block a queue on an
            # in-flight collective.  One instruction per slab.
            def v_readback_slab(s):
                g, cc = divmod(s, n_cc)
                nc.sync.dma_start(
                    v_sb[:, :, g * hh + cc * IC:g * hh + (cc + 1) * IC],
                    vago[cc][g * seq:(g + 1) * seq, :].rearrange(
                        "(j p) c -> p j c", p=P))

            for cg in range(n_cg):
                if cg + 2 < n_cg:
                    load_wg(cg + 2)
                wg = wg_tiles.pop(cg)
                if cg == n_cg - 2:
                    v_readback_slab(0)
                if cg == n_cg - 1:
                    v_readback_slab(1)
                for cl in range(CG):
                    ct = cg * CG + cl
                    for ic in range(n_oc):
                        isl = slice(ic * IC, (ic + 1) * IC)
                        ps = psump.tile([P, IC], F32, tag="ps", name="ps")
                        for kk in range(0, nd, 2):
                            nc.tensor.matmul(ps[:],
                                             wg[:, kk:kk + 2,
                                                cl * P:(cl + 1) * P],
                                             xTo_sb[:, ic, kk:kk + 2, :],
                                             start=(kk == 0),
                                             stop=(kk == nd - 2),
                                             perf_mode=DR)
                        # gate = silu(psum/SH + b), fp8 at true scale
                        nc.scalar.activation(gt_sb[:, ct, isl], ps[:],
                                             AF.Silu,
                                             bias=bhgT_sb[:, ct:ct + 1],
                                             scale=1.0 / SH)

        # ---- attention output + final projection (own rows only) ----
        with tc.tile_pool(name="ph2", bufs=1) as ph2p:
            og_sb = [ph2p.tile([P, nctg, IC], FP8, tag=f"og{i}",
                               name=f"og{i}") for i in range(n_oc)]
            # ogT[all c, chunk] = (v^T @ attnT) * gateT, both chunks first
            # (both og buffers stay live so the out-projection can then run
            # dc-outer across chunks, loading each Wout column-block ONCE)
            n_slab = 2 * n_cc            # v column slabs
            ctps = nctg // n_slab        # og c-tiles per slab
            for ic in range(n_oc):
                isl = slice(ic * IC, (ic + 1) * IC)
                og = og_sb[ic]
                for ct in range(nctg):
                    if ic == 0 and ct % ctps == 0 and ct // ctps + 2 < n_slab:
                        v_readback_slab(ct // ctps + 2)
                    ps = psump.tile([P, IC], F32, tag="ps", name="ps")
                    for kk in range(0, njt, 2):
                        nc.tensor.matmul(ps[:],
                                         v_sb[:, kk:kk + 2, ct * P:(ct + 1) * P],
                                         at_sb[:, kk:kk + 2, isl],
                                         start=(kk == 0), stop=(kk == njt - 2),
                                         perf_mode=DR)
                    nc.vector.tensor_tensor(og[:, ct, :], ps[:],
                                            gt_sb[:, ct, isl], ALU.mult)
            # final rows: out[own rows, :] = POSC ogT^T Wout + xres
            for dc in range(n_dc):
                wo = ph2p.tile([P, nctg, DC], FP8, tag="wo", bufs=2,
                              name="wo")
                nc.gpsimd.dma_start(wo[:], wout_d[dc * P:(dc + 1) * P])
                # all residual rows for this column block load up front
                # (split over two queues) so the write-out chain never
                # waits on them -- the late-xr wait used to stretch the
                # end-of-kernel drain by ~8us
                xrs = []
                for t in range(n_oc * n_it):
                    xr = ph2p.tile([P, DC], F32, tag="xr",
                                   bufs=n_oc * n_it - 2, name="xr")
                    q = nc.sync if t % 2 else nc.scalar
                    q.dma_start(xr[:],
                                xres_d[t * P:(t + 1) * P,
                                       dc * DC:(dc + 1) * DC])
                    xrs.append(xr)
                for ic in range(n_oc):
                    for it in range(n_it):
                        orow = ic * IC + it * P
                        xr = xrs[ic * n_it + it]
                        ps = psump.tile([P, DC], F32, tag="ps", name="ps")
                        for kk in range(0, nctg, 2):
                            nc.tensor.matmul(ps[:],
                                             og_sb[ic][:, kk:kk + 2,
                                                       it * P:(it + 1) * P],
                                             wo[:, kk:kk + 2, :],
                                             start=(kk == 0),
                                             stop=(kk == nctg - 2),
                                             perf_mode=DR)
                        po = ph2p.tile([P, DC], F32, tag="po", bufs=2,
                                       name="po")
                        fo = ph2p.tile([P, DC], F32, tag="fo", bufs=2,
                                       name="fo")
                        last = (dc == n_dc - 1 and ic == n_oc - 1
                                and it == n_it - 1)
                        # the very last tile drains in narrow strips so the
                        # end-of-kernel ACT->DVE->DMA chain is short
                        nst = 4 if last else 1
                        sw = DC // nst
                        for st_ in range(nst):
                            ssl = slice(st_ * sw, (st_ + 1) * sw)
                            nc.scalar.mul(po[:, ssl], ps[:, ssl], POSC)
                            nc.vector.tensor_tensor(fo[:, ssl], xr[:, ssl],
                                                    po[:, ssl], ALU.add)
                            wq = (nc.scalar
                                  if (ic * n_it + it + st_) % 2 else nc.sync)
                            wq.dma_start(
                                out_d[orow:orow + P,
                                      dc * DC + st_ * sw:
                                      dc * DC + (st_ + 1) * sw],
                                fo[:, ssl])

    nc.compile()
    return nc


def TileCtx(nc):
    return tile.TileContext(nc)


def own_rows(seq, h, IC_=None):
    """Rows owned by pair-member h: the contiguous h-th half."""
    return np.arange(h * (seq // 2), (h + 1) * (seq // 2))


def _to_fp8(a):
    return np.clip(a, -224.0, 224.0).astype(ml_dtypes.float8_e4m3)


def make_in_maps(x, W_hidden, b_hidden, W_qk, b_qk, gamma_q, beta_q,
                 gamma_k, beta_k, W_out, b_out, n_cores=8):
    """Host-side sharding/layout prep.  Returns per-core input dicts."""
    B, seq, dim = x.shape
    H2 = W_hidden.shape[1]
    H = H2 // 2
    hh = H // 2  # per-core v-half width
    nctg = H // P
    in_maps = []
    xT_cache = {}
    whg8 = _tile_pack(_to_fp8(W_hidden[:, H:] * SH), P, nctg // CG, CG * P)
    wout8 = _tile_pack(_to_fp8(W_out * SO), P, dim // DC, DC)
    wqk8 = _to_fp8(np.ascontiguousarray(
        np.concatenate(np.split(W_qk * SH, dim // P, axis=0), axis=1)))
    bhgT = np.ascontiguousarray(
        b_hidden[H:].reshape(nctg, P).T).astype(np.float32)
    whv8 = {}
    for core in range(n_cores):
        b, h = core // 2, core % 2
        if b not in xT_cache:
            xT8 = _to_fp8(np.ascontiguousarray(x[b].T))
            rows = own_rows(seq, h)
            xT_cache[b] = (
                _tile_pack(xT8, P, seq // IC, IC),
                xT8,
            )
        rows = own_rows(seq, h)
        xres = (x[b][rows].astype(np.float32)
                + b_out.astype(np.float32)[None, :])
        cs = slice(h * hh, (h + 1) * hh)
        if h not in whv8:
            whv8[h] = _tile_pack(_to_fp8(W_hidden[:, cs] * SH),
                                 P, hh // IC, IC)
        in_maps.append({
            "xT": xT_cache[b][0],
            "xTo": _tile_pack(
                np.ascontiguousarray(xT_cache[b][1][:, rows]),
                P, (seq // 2) // IC, IC),
            "whv": whv8[h],
            "whg": whg8,
            "wqk": wqk8,
            "wout": wout8,
            "bqk": b_qk.reshape(-1, 1).astype(np.float32),
            "gq": gamma_q.reshape(-1, 1).astype(np.float32),
            "bq": beta_q.reshape(-1, 1).astype(np.float32),
            "gk": gamma_k.reshape(-1, 1).astype(np.float32),
            "bk": beta_k.reshape(-1, 1).astype(np.float32),
            "bhv": (b_hidden[cs] * SH).reshape(1, -1).astype(
                ml_dtypes.bfloat16),
            "bhgT": bhgT,
            "xres": xres,
        })
    return in_maps


_NC_CACHE = {}


def _get_nc(seq, dim, hh, n_cores, with_bhv=True, fastqk=True):
    key = (seq, dim, hh, n_cores, with_bhv, fastqk)
    if key not in _NC_CACHE:
        _NC_CACHE[key] = build_gau_nc(seq=seq, dim=dim, hh=hh,
                                      n_cores=n_cores, with_bhv=with_bhv,
                                      fastqk=fastqk)
    return _NC_CACHE[key]


def _is_fastqk(gamma_q, beta_q, gamma_k, beta_k):
    return bool(np.all(gamma_q == 1.0) and np.all(beta_q == 0.0)
                and np.all(gamma_k == 1.0) and np.all(beta_k == 0.0))


def kernel(x, W_hidden, b_hidden, W_qk, b_qk, gamma_q, beta_q, gamma_k,
           beta_k, W_out, b_out):
    x = np.asarray(x)
    B, seq, dim = x.shape
    hh = W_hidden.shape[1] // 4
    n_cores = 2 * B
    with_bhv = bool(np.any(np.asarray(b_hidden)[: 2 * hh] != 0))
    fastqk = _is_fastqk(np.asarray(gamma_q), np.asarray(beta_q),
                        np.asarray(gamma_k), np.asarray(beta_k))
    nc = _get_nc(seq, dim, hh, n_cores, with_bhv=with_bhv, fastqk=fastqk)
    in_maps = make_in_maps(x, np.asarray(W_hidden), np.asarray(b_hidden),
                           np.asarray(W_qk), np.asarray(b_qk),
                           np.asarray(gamma_q), np.asarray(beta_q),
                           np.asarray(gamma_k), np.asarray(beta_k),
                           np.asarray(W_out), np.asarray(b_out),
                           n_cores=n_cores)
    res = run_bass_kernel_spmd(nc, in_maps, core_ids=list(range(n_cores)))
    out = np.empty((B, seq, dim), np.float32)
    for b in range(B):
        for h in range(2):
            out[b, own_rows(seq, h)] = res.results[2 * b + h]["out"]
    return out
